# Initial kernel scaffold
#
"""Trainium2 Bass kernel for MinimalKAN forward (nn_MinimalKAN_Normalized).

Math:
  a = sigmoid(alpha)
  out = (1-a) * (x @ W.T + b) + (a/sqrt(I)) * (x @ C0 + x^2 @ C1 + x^3 @ C2)

Folding the alpha blend into the weights on the host gives exactly
  out = x @ A + x^2 @ B + x^3 @ C + b_eff
with A = (1-a) W.T + s C0, B = s C1, C = s C2, b_eff = (1-a) b, s = a/sqrt(I).

Device strategy (data-parallel over batch, 8 cores), per core shard 4096 rows:
  The contraction index i must sit on SBUF partitions for the TensorEngine,
  so the kernel consumes x^T.  Host mode (default) feeds x^T per core and the
  device runs pure matmuls; device mode PE-transposes x tiles via identity.
  Per 512-row batch group:
    - DMA x^T group [128, 4, 512] (f32r)
    - ACT: x2 = Square(x), DVE: x3 = x2*x  (group-batched, f32r)
    - per 128-row tile: 12 accumulating f32r matmuls into one PSUM bank
        (lhsT = basis^T k-slice [128,128], rhs = weight slice [128,512])
    - DVE: bias add fused into the PSUM->SBUF copy
    - DMA out group [128, 4, 512] on the ACT HWDGE ring
Matmul dtype (KAN_MM_DTYPE): float16 default — fp16 weights are host-scaled
by WSCALE=64 to clear the subnormal range and the PSUM result is rescaled in
the fused bias-add; ~2.4e-4 rel error at 1 cyc/row.  float32r: ~1.2e-4 rel
error but ~1.22 cyc/row (moving-operand SBUF bandwidth); float32: exact, 4x
slower.  fp32/f32r mixing with 16-bit operands is rejected by the hardware.
"""

import os
import numpy as np

import concourse.bass as bass
from concourse import bacc
import concourse.mybir as mybir
import concourse.tile as tile
from concourse.bass_utils import run_bass_kernel_spmd
from concourse.masks import make_identity

N_CORES = 8
B, I, O = 32768, 512, 512
BS = B // N_CORES          # rows per core
P = 128
N_TILES = BS // P          # 32 tiles per core
KS = I // P                # 4 contraction slices per basis

_MM_DTYPE = os.environ.get("KAN_MM_DTYPE", "float16")
_HOST_T = os.environ.get("KAN_HOST_T", "1") == "1"
_GROUP = int(os.environ.get("KAN_GROUP", "4"))
_W_BF16 = os.environ.get("KAN_W_BF16", "0") == "1"


WSCALE = 64.0  # host multiplies fp16 weights by this to stay in normal range


def _build(mm_dtype_name: str, repeat: int = 1, host_t: bool = _HOST_T,
           group: int = _GROUP, w_bf16: bool = _W_BF16) -> bass.Bass:
    mm_dt = getattr(mybir.dt, mm_dtype_name)
    w_dt = mybir.dt.bfloat16 if w_bf16 else mm_dt
    if mm_dtype_name in ("float32r", "float16"):
        x_dt = mm_dt
    else:
        x_dt = mybir.dt.float32
    G = group
    GB = G * P                     # batch rows per group
    n_groups = N_TILES // G
    sq = mybir.ActivationFunctionType.Square

    nc = bacc.Bacc("TRN2", target_bir_lowering=False, debug=False,
                   num_devices=N_CORES)

    if host_t:
        x_d = nc.dram_tensor("xt", [I, BS], x_dt, kind="ExternalInput")
        x_r = x_d.rearrange("(ks p) b -> p ks b", p=P)
    else:
        x_d = nc.dram_tensor("x", [BS, I], x_dt, kind="ExternalInput")
        x_g = x_d.rearrange("(g a p) k -> g p a k", a=G, p=P)
    w_d = nc.dram_tensor("wcat", [3 * I, O], w_dt, kind="ExternalInput")
    b_d = nc.dram_tensor("bias", [P, O], mybir.dt.float32,
                         kind="ExternalInput")
    o_d = nc.dram_tensor("out", [BS, O], mybir.dt.float32,
                         kind="ExternalOutput")
    o_g = o_d.rearrange("(g a p) k -> g p a k", a=G, p=P)

    w_r = w_d.rearrange("(ks p) o -> p ks o", p=P)

    with tile.TileContext(nc) as tc:
        with (
            tc.tile_pool(name="const", bufs=1) as const,
            tc.tile_pool(name="xin", bufs=3) as xin,
            tc.tile_pool(name="xt", bufs=4) as xt,
            tc.tile_pool(name="outp", bufs=4) as outp,
            tc.tile_pool(name="psum_t", bufs=3, space="PSUM") as psum_t,
            tc.tile_pool(name="psum_o", bufs=6, space="PSUM") as psum_o,
        ):
            if not host_t:
                if x_dt == mybir.dt.float32:
                    ident = const.tile([P, P], x_dt)
                    make_identity(nc, ident[:])
                else:
                    ident_f32 = const.tile([P, P], mybir.dt.float32)
                    make_identity(nc, ident_f32[:])
                    ident = const.tile([P, P], x_dt)
                    nc.vector.tensor_copy(out=ident[:], in_=ident_f32[:])

            wsb = const.tile([P, 3 * KS, O], w_dt)
            for ws in range(3 * KS):
                nc.sync.dma_start(wsb[:, ws, :], w_r[:, ws, :])
            bsb = const.tile([P, O], mybir.dt.float32)
            nc.sync.dma_start(bsb[:], b_d[:, :])

            for g in [i for _ in range(repeat) for i in range(n_groups)]:
                if host_t:
                    xT = xt.tile([P, KS, GB], mm_dt, tag="xT")
                    nc.sync.dma_start(xT[:], x_r[:, :, g * GB:(g + 1) * GB])
                else:
                    x_sb = xin.tile([P, G, I], x_dt, tag="x_sb")
                    nc.sync.dma_start(x_sb[:], x_g[g])
                    xT = xt.tile([P, KS, GB], mm_dt, tag="xT")
                    for j in range(G):
                        pt = psum_t.tile([P, KS, P], x_dt, tag="pt")
                        for k in range(KS):
                            nc.tensor.transpose(
                                pt[:, k, :],
                                x_sb[:, j, k * P:(k + 1) * P], ident[:])
                        for k in range(KS):
                            nc.vector.tensor_copy(
                                out=xT[:, k, j * P:(j + 1) * P],
                                in_=pt[:, k, :])

                x2T = xt.tile([P, KS, GB], mm_dt, tag="x2T")
                x3T = xt.tile([P, KS, GB], mm_dt, tag="x3T")
                o_sb = outp.tile([P, G, O], mybir.dt.float32, tag="o_sb")
                PING = os.environ.get("KAN_PINGPONG", "0") == "1"
                for j0 in range(0, G, 2 if PING else 1):
                    jset = [j0, j0 + 1] if PING else [j0]
                    pos = []
                    for j in jset:
                        js = slice(j * P, (j + 1) * P)
                        nc.scalar.activation(x2T[:, :, js], xT[:, :, js], sq)
                        nc.vector.tensor_mul(x3T[:, :, js], x2T[:, :, js],
                                             xT[:, :, js])
                        po_t = psum_o.tile([P, O], mybir.dt.float32,
                                           tag="po", name="po")
                        pos.append(po_t)
                    idx = 0
                    for bi, XT in enumerate((xT, x2T, x3T)):
                        for k in range(KS):
                            for j, po in zip(jset, pos):
                                nc.tensor.matmul(
                                    po[:],
                                    XT[:, k, j * P:(j + 1) * P],
                                    wsb[:, bi * KS + k, :],
                                    start=(idx == 0),
                                    stop=(idx == 3 * KS - 1),
                                    skip_group_check=True,
                                )
                            idx += 1
                    for j, po in zip(jset, pos):
                        if mm_dtype_name == "float16":
                            nc.vector.scalar_tensor_tensor(
                                o_sb[:, j, :], po[:], 1.0 / WSCALE, bsb[:],
                                mybir.AluOpType.mult, mybir.AluOpType.add)
                        else:
                            nc.vector.tensor_add(o_sb[:, j, :], po[:],
                                                 bsb[:])
                if os.environ.get("KAN_SKIP_OUT", "0") != "1":
                    nc.scalar.dma_start(o_g[g], o_sb[:])

    nc.compile()
    return nc


_NC_CACHE: dict[str, bass.Bass] = {}


def _get_nc() -> bass.Bass:
    nc = _NC_CACHE.get(_MM_DTYPE)
    if nc is None:
        nc = _build(_MM_DTYPE)
        _NC_CACHE[_MM_DTYPE] = nc
    return nc


def _fold_weights(coeffs, W, b, alpha):
    a = 1.0 / (1.0 + np.exp(-np.float64(alpha)))
    s = a / np.sqrt(np.float64(I))
    A = (1.0 - a) * W.astype(np.float64).T + s * coeffs[:, :, 0].astype(np.float64)
    Bm = s * coeffs[:, :, 1].astype(np.float64)
    Cm = s * coeffs[:, :, 2].astype(np.float64)
    wcat = np.ascontiguousarray(
        np.concatenate([A, Bm, Cm], axis=0).astype(np.float32))
    b_eff = ((1.0 - a) * b.astype(np.float64)).astype(np.float32)
    bias_rep = np.ascontiguousarray(
        np.broadcast_to(b_eff[None, :], (P, O)).astype(np.float32))
    return wcat, bias_rep


def _make_in_maps(x, coeffs, W, b, alpha):
    wcat, bias_rep = _fold_weights(coeffs, W, b, alpha)
    if _MM_DTYPE == "bfloat16" or _W_BF16:
        import ml_dtypes
        wcat = wcat.astype(ml_dtypes.bfloat16)
    elif _MM_DTYPE == "float16":
        wcat = (wcat.astype(np.float64) * WSCALE).astype(np.float16)
    x = np.asarray(x, dtype=np.float32)
    in_maps = []
    for c in range(N_CORES):
        shard = x[c * BS:(c + 1) * BS]
        m = {"wcat": wcat, "bias": bias_rep}
        x_np = np.float16 if _MM_DTYPE == "float16" else np.float32
        if _HOST_T:
            m["xt"] = np.ascontiguousarray(shard.T.astype(x_np))
        else:
            m["x"] = np.ascontiguousarray(shard.astype(x_np))
        in_maps.append(m)
    return in_maps


def _run(x, coeffs, W, b, alpha, trace=False):
    nc = _get_nc()
    in_maps = _make_in_maps(x, coeffs, W, b, alpha)
    res = run_bass_kernel_spmd(nc, in_maps, core_ids=list(range(N_CORES)),
                               trace=trace)
    out = np.concatenate([r["out"] for r in res.results], axis=0)
    return out, res


def kernel(x, coeffs, W, b, alpha):
    out, _ = _run(x, coeffs, W, b, alpha, trace=False)
    return out



# revision 1
# speedup vs baseline: 1.0629x; 1.0629x over previous
"""Trainium2 Bass kernel for MinimalKAN forward (nn_MinimalKAN_Normalized).

Math:
  a = sigmoid(alpha)
  out = (1-a) * (x @ W.T + b) + (a/sqrt(I)) * (x @ C0 + x^2 @ C1 + x^3 @ C2)

Folding the alpha blend into the weights on the host gives exactly
  out = x @ A + x^2 @ B + x^3 @ C + b_eff
with A = (1-a) W.T + s C0, B = s C1, C = s C2, b_eff = (1-a) b, s = a/sqrt(I).

Device strategy (data-parallel over batch, 8 cores), per core shard 4096 rows:
  The contraction index i must sit on SBUF partitions for the TensorEngine,
  so the kernel consumes x^T.  Host mode (default) feeds x^T per core and the
  device runs pure matmuls; device mode PE-transposes x tiles via identity.
  Per 512-row batch group:
    - DMA x^T group [128, 4, 512] (f32r)
    - ACT: x2 = Square(x), DVE: x3 = x2*x  (group-batched, f32r)
    - per 128-row tile: 12 accumulating f32r matmuls into one PSUM bank
        (lhsT = basis^T k-slice [128,128], rhs = weight slice [128,512])
    - DVE: bias add fused into the PSUM->SBUF copy
    - DMA out group [128, 4, 512] on the ACT HWDGE ring
Matmul dtype (KAN_MM_DTYPE): float16 default — fp16 weights are host-scaled
by WSCALE=64 to clear the subnormal range and the PSUM result is rescaled in
the fused bias-add; ~2.4e-4 rel error at 1 cyc/row.  float32r: ~1.2e-4 rel
error but ~1.22 cyc/row (moving-operand SBUF bandwidth); float32: exact, 4x
slower.  fp32/f32r mixing with 16-bit operands is rejected by the hardware.
"""

import os
import numpy as np

import concourse.bass as bass
from concourse import bacc
import concourse.mybir as mybir
import concourse.tile as tile
from concourse.bass_utils import run_bass_kernel_spmd
from concourse.masks import make_identity

N_CORES = 8
B, I, O = 32768, 512, 512
BS = B // N_CORES          # rows per core
P = 128
N_TILES = BS // P          # 32 tiles per core
KS = I // P                # 4 contraction slices per basis

_MM_DTYPE = os.environ.get("KAN_MM_DTYPE", "float16")
_HOST_T = os.environ.get("KAN_HOST_T", "1") == "1"
_GROUP = int(os.environ.get("KAN_GROUP", "4"))
_W_BF16 = os.environ.get("KAN_W_BF16", "0") == "1"


WSCALE = 64.0  # host multiplies fp16 weights by this to stay in normal range


def _build(mm_dtype_name: str, repeat: int = 1, host_t: bool = _HOST_T,
           group: int = _GROUP, w_bf16: bool = _W_BF16) -> bass.Bass:
    mm_dt = getattr(mybir.dt, mm_dtype_name)
    w_dt = mybir.dt.bfloat16 if w_bf16 else mm_dt
    if mm_dtype_name in ("float32r", "float16"):
        x_dt = mm_dt
    else:
        x_dt = mybir.dt.float32
    G = group
    GB = G * P                     # batch rows per group
    n_groups = N_TILES // G
    sq = mybir.ActivationFunctionType.Square

    nc = bacc.Bacc("TRN2", target_bir_lowering=False, debug=False,
                   num_devices=N_CORES)

    if host_t:
        x_d = nc.dram_tensor("xt", [I, BS], x_dt, kind="ExternalInput")
        x_r = x_d.rearrange("(ks p) b -> p ks b", p=P)
    else:
        x_d = nc.dram_tensor("x", [BS, I], x_dt, kind="ExternalInput")
        x_g = x_d.rearrange("(g a p) k -> g p a k", a=G, p=P)
    w_d = nc.dram_tensor("wcat", [3 * I, O], w_dt, kind="ExternalInput")
    b_d = nc.dram_tensor("bias", [P, O], mybir.dt.float32,
                         kind="ExternalInput")
    o_d = nc.dram_tensor("out", [BS, O], mybir.dt.float32,
                         kind="ExternalOutput")
    o_g = o_d.rearrange("(g a p) k -> g p a k", a=G, p=P)

    w_r = w_d.rearrange("(ks p) o -> p ks o", p=P)

    with tile.TileContext(nc) as tc:
        with (
            tc.tile_pool(name="const", bufs=1) as const,
            tc.tile_pool(name="xin", bufs=3) as xin,
            tc.tile_pool(name="xt", bufs=4) as xt,
            tc.tile_pool(name="outp", bufs=4) as outp,
            tc.tile_pool(name="psum_t", bufs=3, space="PSUM") as psum_t,
            tc.tile_pool(name="psum_o", bufs=6, space="PSUM") as psum_o,
        ):
            if not host_t:
                if x_dt == mybir.dt.float32:
                    ident = const.tile([P, P], x_dt)
                    make_identity(nc, ident[:])
                else:
                    ident_f32 = const.tile([P, P], mybir.dt.float32)
                    make_identity(nc, ident_f32[:])
                    ident = const.tile([P, P], x_dt)
                    nc.vector.tensor_copy(out=ident[:], in_=ident_f32[:])

            wsb = const.tile([P, 3 * KS, O], w_dt)
            for ws in range(3 * KS):
                nc.sync.dma_start(wsb[:, ws, :], w_r[:, ws, :])
            bsb = const.tile([P, O], mybir.dt.float32)
            nc.sync.dma_start(bsb[:], b_d[:, :])

            for g in [i for _ in range(repeat) for i in range(n_groups)]:
                if host_t:
                    xT = xt.tile([P, KS, GB], mm_dt, tag="xT")
                    nc.sync.dma_start(xT[:], x_r[:, :, g * GB:(g + 1) * GB])
                else:
                    x_sb = xin.tile([P, G, I], x_dt, tag="x_sb")
                    nc.sync.dma_start(x_sb[:], x_g[g])
                    xT = xt.tile([P, KS, GB], mm_dt, tag="xT")
                    for j in range(G):
                        pt = psum_t.tile([P, KS, P], x_dt, tag="pt")
                        for k in range(KS):
                            nc.tensor.transpose(
                                pt[:, k, :],
                                x_sb[:, j, k * P:(k + 1) * P], ident[:])
                        for k in range(KS):
                            nc.vector.tensor_copy(
                                out=xT[:, k, j * P:(j + 1) * P],
                                in_=pt[:, k, :])

                x2T = xt.tile([P, KS, GB], mm_dt, tag="x2T")
                x3T = xt.tile([P, KS, GB], mm_dt, tag="x3T")
                o_sb = outp.tile([P, G, O], mybir.dt.float32, tag="o_sb")
                PING = os.environ.get("KAN_PINGPONG", "0") == "1"
                for j0 in range(0, G, 2 if PING else 1):
                    jset = [j0, j0 + 1] if PING else [j0]
                    pos = []
                    for j in jset:
                        js = slice(j * P, (j + 1) * P)
                        nc.scalar.activation(x2T[:, :, js], xT[:, :, js], sq)
                        nc.vector.tensor_mul(x3T[:, :, js], x2T[:, :, js],
                                             xT[:, :, js])
                        po_t = psum_o.tile([P, O], mybir.dt.float32,
                                           tag="po", name="po")
                        pos.append(po_t)
                    idx = 0
                    for bi, XT in enumerate((xT, x2T, x3T)):
                        for k in range(KS):
                            for j, po in zip(jset, pos):
                                nc.tensor.matmul(
                                    po[:],
                                    XT[:, k, j * P:(j + 1) * P],
                                    wsb[:, bi * KS + k, :],
                                    start=(idx == 0),
                                    stop=(idx == 3 * KS - 1),
                                    skip_group_check=True,
                                )
                            idx += 1
                    for j, po in zip(jset, pos):
                        if mm_dtype_name == "float16":
                            nc.vector.scalar_tensor_tensor(
                                o_sb[:, j, :], po[:], 1.0 / WSCALE, bsb[:],
                                mybir.AluOpType.mult, mybir.AluOpType.add)
                        else:
                            nc.vector.tensor_add(o_sb[:, j, :], po[:],
                                                 bsb[:])
                if os.environ.get("KAN_SKIP_OUT", "0") != "1":
                    nc.scalar.dma_start(o_g[g], o_sb[:])

    nc.compile()
    return nc


_NC_CACHE: dict[str, bass.Bass] = {}


def _get_nc() -> bass.Bass:
    nc = _NC_CACHE.get(_MM_DTYPE)
    if nc is None:
        nc = _build(_MM_DTYPE)
        _NC_CACHE[_MM_DTYPE] = nc
    return nc


def _fold_weights(coeffs, W, b, alpha):
    a = 1.0 / (1.0 + np.exp(-np.float64(alpha)))
    s = a / np.sqrt(np.float64(I))
    A = (1.0 - a) * W.astype(np.float64).T + s * coeffs[:, :, 0].astype(np.float64)
    Bm = s * coeffs[:, :, 1].astype(np.float64)
    Cm = s * coeffs[:, :, 2].astype(np.float64)
    wcat = np.ascontiguousarray(
        np.concatenate([A, Bm, Cm], axis=0).astype(np.float32))
    b_eff = ((1.0 - a) * b.astype(np.float64)).astype(np.float32)
    bias_rep = np.ascontiguousarray(
        np.broadcast_to(b_eff[None, :], (P, O)).astype(np.float32))
    return wcat, bias_rep


def _make_in_maps(x, coeffs, W, b, alpha):
    wcat, bias_rep = _fold_weights(coeffs, W, b, alpha)
    if _MM_DTYPE == "bfloat16" or _W_BF16:
        import ml_dtypes
        wcat = wcat.astype(ml_dtypes.bfloat16)
    elif _MM_DTYPE == "float16":
        wcat = (wcat.astype(np.float64) * WSCALE).astype(np.float16)
    x = np.asarray(x, dtype=np.float32)
    in_maps = []
    for c in range(N_CORES):
        shard = x[c * BS:(c + 1) * BS]
        m = {"wcat": wcat, "bias": bias_rep}
        x_np = np.float16 if _MM_DTYPE == "float16" else np.float32
        if _HOST_T:
            m["xt"] = np.ascontiguousarray(shard.T.astype(x_np))
        else:
            m["x"] = np.ascontiguousarray(shard.astype(x_np))
        in_maps.append(m)
    return in_maps


def _run(x, coeffs, W, b, alpha, trace=False):
    nc = _get_nc()
    in_maps = _make_in_maps(x, coeffs, W, b, alpha)
    res = run_bass_kernel_spmd(nc, in_maps, core_ids=list(range(N_CORES)),
                               trace=trace)
    out = np.concatenate([r["out"] for r in res.results], axis=0)
    return out, res


def kernel(x, coeffs, W, b, alpha):
    out, _ = _run(x, coeffs, W, b, alpha, trace=False)
    return out

